# revision 2
# baseline (speedup 1.0000x reference)
"""GRU Trainium kernel v14b (exact tanh on Act). Single group per core (b=32), minimal-stall cycle:

  per step t (PSUM slots in two dedicated banks, zero-region correct):
    PE:   prz += Wrz@b ; pnh += Wn@b      (b = z*h ready early)
    PE:   prz += Wrz@a ; pnh += Wn@a      (a = (1-z)*n from prev step tail)
    PE:   2 sliced gx-prefetch matmuls for the NEXT chunk's slot t
    Act:  rz = sigmoid(prz)                (PSUM->SBUF)   } parallel legs
    DVE:  ps = copy(pn|pnh)                (PSUM->SBUF)   }
    Pool: u = r*ghn; s = u+gxn; n = tanh13(s); a' = w*n   (in-order run)
    Pool off-path: w = 1-z; q = w*h; b' = h-q; h' = a+b'  (lazy)

tanh13: clamped odd deg-13 polynomial (end-to-end GRU err ~4e-3 << 2e-2 tol).
W@h' is fed as W@a' + W@b' (h' itself off the critical path). b_hn rides the
a-tile ones row; other biases ride xt's ones channel. gx chunk matmuls are
sliced per-step and prefetched one chunk ahead so no 400ns+ matmul ever blocks
the in-order PE stream. All SB+SB operand pairs share base partition 0.
"""
import sys

sys.path.insert(0, "/opt/trn_rl_repo")
import numpy as np
from contextlib import ExitStack

import concourse.bass as bass
import concourse.bacc as bacc
import concourse.tile as tile
from concourse import mybir
from concourse.bass_utils import run_bass_kernel_spmd

F32 = mybir.dt.float32
AF = mybir.ActivationFunctionType
ALU = mybir.AluOpType

N_CORES = 8
B_FULL, T_FULL, H = 256, 2048, 50
B = B_FULL // N_CORES
K = H + 1
ZOFF = 64

G = 1
b = B
Tc = 8
N2 = Tc * 2 * b  # padded chunk width (real|zero column pairs)
STEPS_PER_CHUNK = Tc

TANH_QUADS = [(-36.75023270699887, 351.56895886705087),
              (-20.35104786296833, 167.96545857262166),
              (-1.6698613951205863, 33.55532712841415)]
TANH_LEAD = 4.958407706355163e-07


def _build(C, repeats=1):
    nc = bacc.Bacc("TRN2", target_bir_lowering=False, debug=False,
                   num_devices=N_CORES)
    xt = nc.dram_tensor("xt", (C, K, N2), F32, kind="ExternalInput")
    wxrz = nc.dram_tensor("wxrz", (K, 128), F32, kind="ExternalInput")
    wxn = nc.dram_tensor("wxn", (K, H), F32, kind="ExternalInput")
    whrz_ab = nc.dram_tensor("whrz_ab", (128, 128), F32,
                             kind="ExternalInput")
    whn_ab = nc.dram_tensor("whn_ab", (128, H), F32, kind="ExternalInput")
    ab0 = nc.dram_tensor("ab0", (K, b), F32, kind="ExternalInput")
    y = nc.dram_tensor("y", (H, b), F32, kind="ExternalOutput")

    with ExitStack() as ctx:
        tc_ctx = ctx.enter_context(tile.TileContext(nc))
        consts = ctx.enter_context(tc_ctx.tile_pool(name="consts", bufs=1))
        xpool = ctx.enter_context(tc_ctx.tile_pool(name="xp", bufs=3))
        przpool = ctx.enter_context(
            tc_ctx.tile_pool(name="przp", bufs=2, space="PSUM"))
        pnpool = ctx.enter_context(
            tc_ctx.tile_pool(name="pnp", bufs=2, space="PSUM"))
        tmps = ctx.enter_context(tc_ctx.tile_pool(name="tmps", bufs=4))

        wxrz_sb = consts.tile([K, 128], F32, tag="wxrz")
        wxn_sb = consts.tile([K, H], F32, tag="wxn")
        whrzab_sb = consts.tile([128, 128], F32, tag="whrzab")
        whnab_sb = consts.tile([128, H], F32, tag="whnab")
        nc.sync.dma_start(out=wxrz_sb[:], in_=wxrz[:, :])
        nc.sync.dma_start(out=wxn_sb[:], in_=wxn[:, :])
        nc.sync.dma_start(out=whrzab_sb[:], in_=whrz_ab[:, :])
        nc.sync.dma_start(out=whnab_sb[:], in_=whn_ab[:, :])

        h_bufs = [consts.tile([H, b], F32, tag=f"h_{i}", name=f"h_{i}")
                  for i in range(3)]
        # ab state tile: a rows 0..50 (ones row 50), b rows 64..113;
        # rows 51..63 and 114..127 are junk killed by zero stationary rows
        ab_bufs = [consts.tile([128, b], F32, tag=f"ab_{i}", name=f"ab_{i}")
                   for i in range(3)]
        b_bufs = [consts.tile([H, b], F32, tag=f"b_{i}", name=f"b_{i}")
                  for i in range(3)]
        for _rep in range(repeats):
            for ab in ab_bufs:
                nc.gpsimd.memset(ab[:], 0.0)
                nc.sync.dma_start(out=ab[0:K, :], in_=ab0[:, :])
            for bb in b_bufs:
                nc.gpsimd.memset(bb[:], 0.0)
            for hb in h_bufs:
                nc.gpsimd.memset(hb[:], 0.0)

            # prologue: chunk 0 gx via two full-width matmuls
            xt_sb = [None, None]  # ping-pong SBUF xt chunks
            prz_t = [None, None]
            pn_t = [None, None]
            xt_sb[0] = xpool.tile([K, N2], F32, tag="xt0", name="xt_c0")
            nc.sync.dma_start(out=xt_sb[0][:], in_=xt[0, :, :])
            prz_t[0] = przpool.tile([128, N2], F32, tag="prz", name="prz_c0")
            pn_t[0] = pnpool.tile([128, N2], F32, tag="pn", name="pn_c0")
            nc.tensor.matmul(prz_t[0][:, :], wxrz_sb[:], xt_sb[0][:],
                             start=True, stop=False, skip_group_check=True)
            nc.tensor.matmul(pn_t[0][0:H, :], wxn_sb[:], xt_sb[0][:],
                             start=True, stop=False, skip_group_check=True)

            for c in range(C):
                cur = c % 2
                nxt = (c + 1) % 2
                if c + 1 < C:
                    xt_sb[nxt] = xpool.tile([K, N2], F32, tag=f"xt{nxt}",
                                            name=f"xt{nxt}")
                    nc.sync.dma_start(out=xt_sb[nxt][:], in_=xt[c + 1, :, :])
                    prz_t[nxt] = przpool.tile([128, N2], F32, tag="prz",
                                              name="prz_n")
                    pn_t[nxt] = pnpool.tile([128, N2], F32, tag="pn",
                                            name="pn_n")
                for ti in range(Tc):
                    t = c * Tc + ti
                    lo = 2 * b * ti
                    abc = ab_bufs[t % 3]
                    prz_sl = prz_t[cur][:, lo:lo + b]
                    pn_sl = pn_t[cur][0:H, lo:lo + b]
                    pnh_sl = pn_t[cur][0:H, lo + b:lo + 2 * b]
                    ps_src = pn_t[cur][0:H, lo:lo + 2 * b]
                    nc.tensor.matmul(prz_sl, whrzab_sb[:], abc[:],
                                     start=False, stop=False,
                                     skip_group_check=True)
                    nc.tensor.matmul(pnh_sl, whnab_sb[:], abc[:],
                                     start=False, stop=False,
                                     skip_group_check=True)
                    # gx prefetch slices for next chunk's slot ti
                    if c + 1 < C:
                        xsl = xt_sb[nxt][:, lo:lo + 2 * b]
                        nc.tensor.matmul(prz_t[nxt][:, lo:lo + 2 * b],
                                         wxrz_sb[:], xsl,
                                         start=(ti == 0), stop=False,
                                         skip_group_check=True)
                        nc.tensor.matmul(pn_t[nxt][0:H, lo:lo + 2 * b],
                                         wxn_sb[:], xsl,
                                         start=(ti == 0), stop=False,
                                         skip_group_check=True)
                    rz = tmps.tile([128, b], F32, tag="rz", name="rz")
                    nc.scalar.activation(rz[:], prz_sl, AF.Sigmoid)
                    ps = tmps.tile([H, 2 * b], F32, tag="ps", name="ps")
                    nc.vector.tensor_copy(ps[:], ps_src)
                    # Pool in-order tail
                    abn = ab_bufs[(t + 1) % 3]
                    u = tmps.tile([H, b], F32, tag="u", name="u")
                    nc.gpsimd.tensor_mul(u[:], rz[0:H, :], ps[:, b:2 * b])
                    s2 = tmps.tile([H, b], F32, tag="s", name="s")
                    nc.gpsimd.tensor_add(s2[:], u[:], ps[:, 0:b])
                    w = tmps.tile([H, b], F32, tag="w", name="w")
                    nc.vector.tensor_scalar(w[:], rz[ZOFF:ZOFF + H, :],
                                            -1.0, 1.0, ALU.mult, ALU.add)
                    n_sb = tmps.tile([H, b], F32, tag="n", name="n")
                    nc.scalar.activation(n_sb[:], s2[:], AF.Tanh)
                    nc.gpsimd.tensor_mul(abn[0:H, :], w[:], n_sb[:])
                    # off-path: b' = h - w*h kept at base 0 (bn) and copied
                    # into the ab tile rows 64..113 for the combined matmul
                    hc = h_bufs[t % 3]
                    hn = h_bufs[(t + 1) % 3]
                    bn = b_bufs[(t + 1) % 3]
                    with tc_ctx.high_priority(offset=-64):
                        q = tmps.tile([H, b], F32, tag="q", name="q")
                        nc.vector.tensor_mul(q[:], w[:], hc[:])
                        nc.vector.tensor_sub(bn[:], hc[:], q[:])
                        nc.vector.tensor_copy(abn[ZOFF:ZOFF + H, :], bn[:])
                        nc.vector.tensor_add(hn[:], abn[0:H, :], bn[:])
        h_final = h_bufs[(C * Tc) % 3]
        nc.sync.dma_start(out=y[:, :], in_=h_final[:, :])
    nc.compile()
    return nc


def _build_nc(repeats=1):
    return _build(T_FULL // Tc, repeats)


def build_probe(C):
    return _build(C)


def _prep_in_maps(inputs, W_ih, W_hh, b_ih, b_hh):
    C = T_FULL // Tc
    inputs = np.ascontiguousarray(inputs, dtype=np.float32)
    W_ih = np.asarray(W_ih, dtype=np.float32)
    W_hh = np.asarray(W_hh, dtype=np.float32)
    b_ih = np.asarray(b_ih, dtype=np.float32)
    b_hh = np.asarray(b_hh, dtype=np.float32)

    wxrz = np.zeros((K, 128), np.float32)
    wxrz[0:H, 0:H] = W_ih[0:H].T
    wxrz[0:H, ZOFF:ZOFF + H] = W_ih[H:2 * H].T
    wxrz[H, 0:H] = b_ih[0:H] + b_hh[0:H]
    wxrz[H, ZOFF:ZOFF + H] = b_ih[H:2 * H] + b_hh[H:2 * H]
    wxn = np.empty((K, H), np.float32)
    wxn[0:H] = W_ih[2 * H:3 * H].T
    wxn[H] = b_ih[2 * H:3 * H]

    whrz = np.zeros((H, 128), np.float32)
    whrz[0:H, 0:H] = W_hh[0:H].T
    whrz[0:H, ZOFF:ZOFF + H] = W_hh[H:2 * H].T
    whn = W_hh[2 * H:3 * H].T.astype(np.float32)
    whrz_ab = np.zeros((128, 128), np.float32)
    whrz_ab[0:H] = whrz
    whrz_ab[ZOFF:ZOFF + H] = whrz
    whn_ab = np.zeros((128, H), np.float32)
    whn_ab[0:H] = whn
    whn_ab[H] = b_hh[2 * H:3 * H]
    whn_ab[ZOFF:ZOFF + H] = whn
    ab0 = np.zeros((K, b), np.float32)
    ab0[H] = 1.0

    in_maps = []
    for core in range(N_CORES):
        xc = inputs[core * B:(core + 1) * B]
        xa = np.concatenate([xc, np.ones((B, T_FULL, 1), np.float32)], axis=2)
        xg = xa.reshape(b, C, Tc, K).transpose(1, 3, 2, 0)  # C,K,Tc,b
        xt2 = np.zeros((C, K, Tc, 2, b), np.float32)
        xt2[:, :, :, 0, :] = xg
        xtv = np.ascontiguousarray(xt2.reshape(C, K, N2))
        in_maps.append({"xt": xtv, "wxrz": wxrz, "wxn": wxn,
                        "whrz_ab": whrz_ab, "whn_ab": whn_ab, "ab0": ab0})
    return in_maps


_NC_CACHE = []


def kernel(inputs, W_ih, W_hh, b_ih, b_hh, z=0, **_ignored):
    if np.asarray(inputs).ndim == 2:
        inputs = np.asarray(inputs)[None]
    if not _NC_CACHE:
        _NC_CACHE.append(_build_nc())
    nc = _NC_CACHE[0]
    in_maps = _prep_in_maps(inputs, W_ih, W_hh, b_ih, b_hh)
    res = run_bass_kernel_spmd(nc, in_maps, core_ids=list(range(N_CORES)))
    out = np.empty((B_FULL, H), np.float32)
    for core in range(N_CORES):
        out[core * B:(core + 1) * B] = res.results[core]["y"].T
    return out


# revision 3
# speedup vs baseline: 14.3618x; 14.3618x over previous
"""Trainium2 Bass kernel for a single-layer GRU (PyTorch semantics), returning
the final hidden state h_T.

Problem: inputs (256, 2048, 50) fp32, W_ih/W_hh (150, 50), b_ih/b_hh (150,).
Strategy: data-parallel over 8 NeuronCores (32 sequences each). Per core the
recurrence runs in a transposed layout (hidden units on SBUF partitions, batch
on the free dimension). The input projection gx = W_ih @ x_t^T for 16 time
steps at a time is computed by one PE matmul into a PSUM bank (start=True) and
each step's recurrent matmul accumulates its gh contribution into the same
bank slice (start=False) — gx is never materialized to HBM. Biases are folded
via an appended ones-channel on x and a ones-row on the h state tile.
"""
import sys

sys.path.insert(0, "/opt/trn_rl_repo")
import numpy as np
from contextlib import ExitStack

import concourse.bass as bass
import concourse.bacc as bacc
import concourse.tile as tile
from concourse import mybir
from concourse.bass_utils import run_bass_kernel_spmd

F32 = mybir.dt.float32
AF = mybir.ActivationFunctionType

N_CORES = 8
B_FULL, T, H = 256, 2048, 50
B = B_FULL // N_CORES  # 32 sequences per core
Tc = 16  # time steps per PSUM chunk (16*32 = 512 fp32 = one PSUM bank)
C = T // Tc
K = H + 1  # hidden/input dim augmented with a ones row (bias folding)
N = Tc * B
# z gate lives at partition offset 64 inside the padded 128-partition r/z
# block (engine SBUF access must start at a multiple-of-32 partition).
ZOFF = 64


def _build_nc(repeats=1):
    """repeats > 1 reruns the whole computation sequentially inside one NEFF
    (used only by the test harness for wall-clock timing amplification)."""
    nc = bacc.Bacc("TRN2", target_bir_lowering=False, debug=False,
                   num_devices=N_CORES)
    xt = nc.dram_tensor("xt", (C, K, N), F32, kind="ExternalInput")
    wxrz = nc.dram_tensor("wxrz", (K, 128), F32, kind="ExternalInput")
    wxn = nc.dram_tensor("wxn", (K, H), F32, kind="ExternalInput")
    whrz = nc.dram_tensor("whrz", (K, 128), F32, kind="ExternalInput")
    whn = nc.dram_tensor("whn", (K, H), F32, kind="ExternalInput")
    h0init = nc.dram_tensor("h0init", (K, B), F32, kind="ExternalInput")
    y = nc.dram_tensor("y", (H, B), F32, kind="ExternalOutput")

    with ExitStack() as ctx:
        tc_ctx = ctx.enter_context(tile.TileContext(nc))
        consts = ctx.enter_context(tc_ctx.tile_pool(name="consts", bufs=1))
        xpool = ctx.enter_context(tc_ctx.tile_pool(name="xp", bufs=3))
        prz_pool = ctx.enter_context(
            tc_ctx.tile_pool(name="prz", bufs=2, space="PSUM"))
        pn_pool = ctx.enter_context(
            tc_ctx.tile_pool(name="pn", bufs=2, space="PSUM"))
        pnh_pool = ctx.enter_context(
            tc_ctx.tile_pool(name="pnh", bufs=2, space="PSUM"))
        gates = ctx.enter_context(tc_ctx.tile_pool(name="gates", bufs=3))
        tmps = ctx.enter_context(tc_ctx.tile_pool(name="tmps", bufs=3))

        wxrz_sb = consts.tile([K, 128], F32, tag="wxrz")
        wxn_sb = consts.tile([K, H], F32, tag="wxn")
        whrz_sb = consts.tile([K, 128], F32, tag="whrz")
        whn_sb = consts.tile([K, H], F32, tag="whn")
        nc.sync.dma_start(out=wxrz_sb[:], in_=wxrz[:, :])
        nc.sync.dma_start(out=wxn_sb[:], in_=wxn[:, :])
        nc.sync.dma_start(out=whrz_sb[:], in_=whrz[:, :])
        nc.sync.dma_start(out=whn_sb[:], in_=whn[:, :])

        h_bufs = [
            consts.tile([K, B], F32, tag=f"h{i}", name=f"h{i}") for i in range(2)
        ]
        for _rep in range(repeats):
          for hb in h_bufs:
            nc.sync.dma_start(out=hb[:], in_=h0init[:, :])

          for c in range(C):
            xt_sb = xpool.tile([K, N], F32, tag="xt")
            nc.sync.dma_start(out=xt_sb[:], in_=xt[c, :, :])
            prz = prz_pool.tile([128, N], F32, tag="prz")
            pn = pn_pool.tile([H, N], F32, tag="pn")
            nc.tensor.matmul(prz[:], wxrz_sb[:], xt_sb[:], start=True,
                             stop=False, skip_group_check=True)
            nc.tensor.matmul(pn[:], wxn_sb[:], xt_sb[:], start=True,
                             stop=True, skip_group_check=True)
            for ti in range(Tc):
                t = c * Tc + ti
                hc = h_bufs[t % 2]
                hn = h_bufs[(t + 1) % 2]
                sl = bass.ts(ti, B)
                pnh = pnh_pool.tile([H, B], F32, tag="pnh")
                nc.tensor.matmul(prz[:, sl], whrz_sb[:], hc[:], start=False,
                                 stop=True, skip_group_check=True)
                nc.tensor.matmul(pnh[:], whn_sb[:], hc[:], start=True,
                                 stop=True, skip_group_check=True)
                rz = gates.tile([128, B], F32, tag="rz")
                nc.scalar.activation(rz[:], prz[:, sl], AF.Sigmoid)
                u = tmps.tile([H, B], F32, tag="u")
                nc.vector.tensor_mul(u[:], rz[0:H, :], pnh[:])  # r * ghn
                nc.vector.tensor_add(pnh[:], u[:], pn[:, sl])  # + gxn
                n_sb = tmps.tile([H, B], F32, tag="n")
                nc.scalar.activation(n_sb[:], pnh[:], AF.Tanh)
                d = tmps.tile([128, B], F32, tag="d")
                # h - n, written at partition offset ZOFF so the next
                # tensor_tensor sees equal SBUF base partitions
                nc.vector.tensor_sub(d[ZOFF : ZOFF + H, :], hc[0:H, :], n_sb[:])
                yv = tmps.tile([H, B], F32, tag="yv")
                nc.vector.tensor_mul(yv[:], rz[ZOFF : ZOFF + H, :],
                                     d[ZOFF : ZOFF + H, :])
                nc.vector.tensor_add(hn[0:H, :], n_sb[:], yv[:])
        h_final = h_bufs[T % 2]
        nc.sync.dma_start(out=y[:, :], in_=h_final[0:H, :])
    nc.compile()
    return nc


def _prep_in_maps(inputs, W_ih, W_hh, b_ih, b_hh):
    inputs = np.ascontiguousarray(inputs, dtype=np.float32)
    W_ih = np.asarray(W_ih, dtype=np.float32)
    W_hh = np.asarray(W_hh, dtype=np.float32)
    b_ih = np.asarray(b_ih, dtype=np.float32)
    b_hh = np.asarray(b_hh, dtype=np.float32)

    wxrz = np.zeros((K, 128), np.float32)
    wxrz[0:H, 0:H] = W_ih[0:H].T
    wxrz[0:H, ZOFF : ZOFF + H] = W_ih[H : 2 * H].T
    wxrz[H, 0:H] = b_ih[0:H] + b_hh[0:H]
    wxrz[H, ZOFF : ZOFF + H] = b_ih[H : 2 * H] + b_hh[H : 2 * H]
    wxn = np.empty((K, H), np.float32)
    wxn[0:H] = W_ih[2 * H : 3 * H].T
    wxn[H] = b_ih[2 * H : 3 * H]
    whrz = np.zeros((K, 128), np.float32)
    whrz[0:H, 0:H] = W_hh[0:H].T
    whrz[0:H, ZOFF : ZOFF + H] = W_hh[H : 2 * H].T
    whn = np.empty((K, H), np.float32)
    whn[0:H] = W_hh[2 * H : 3 * H].T
    whn[H] = b_hh[2 * H : 3 * H]
    h0init = np.zeros((K, B), np.float32)
    h0init[H] = 1.0

    in_maps = []
    for core in range(N_CORES):
        xc = inputs[core * B : (core + 1) * B]  # (B, T, H)
        xa = np.concatenate([xc, np.ones((B, T, 1), np.float32)], axis=2)
        xt = np.ascontiguousarray(
            xa.reshape(B, C, Tc, K).transpose(1, 3, 2, 0).reshape(C, K, N)
        )
        in_maps.append({"xt": xt, "wxrz": wxrz, "wxn": wxn, "whrz": whrz,
                        "whn": whn, "h0init": h0init})
    return in_maps


_NC_CACHE = []


def kernel(inputs, W_ih, W_hh, b_ih, b_hh, z=0, **_ignored):
    if np.asarray(inputs).ndim == 2:
        inputs = np.asarray(inputs)[None]
    if not _NC_CACHE:
        _NC_CACHE.append(_build_nc())
    nc = _NC_CACHE[0]
    in_maps = _prep_in_maps(inputs, W_ih, W_hh, b_ih, b_hh)
    res = run_bass_kernel_spmd(nc, in_maps, core_ids=list(range(N_CORES)))
    out = np.empty((B_FULL, H), np.float32)
    for core in range(N_CORES):
        out[core * B : (core + 1) * B] = res.results[core]["y"].T
    return out


if __name__ == "__main__":
    rng = np.random.default_rng(0)
    s = 1.0 / np.sqrt(H)
    demo = {
        "inputs": rng.standard_normal((B_FULL, T, H), dtype=np.float32),
        "W_ih": rng.uniform(-s, s, (3 * H, H)).astype(np.float32),
        "W_hh": rng.uniform(-s, s, (3 * H, H)).astype(np.float32),
        "b_ih": rng.uniform(-s, s, (3 * H,)).astype(np.float32),
        "b_hh": rng.uniform(-s, s, (3 * H,)).astype(np.float32),
        "z": 0,
    }
    out = kernel(**demo)
    print("kernel output", out.shape, out.dtype, out[0, :4])

